# revision 27
# baseline (speedup 1.0000x reference)
"""Multi-head dot-product attention (causal) on 8 TRN2 NeuronCores.

Sharding (Megatron-style per hint): batch (2) x head-groups (4 of 4 heads)
= 8 cores. Each core: q/k/v projections for its 4 heads, causal attention,
partial output projection Y_c = sum_h O_h @ Wo_h. Host sums the 4 partials
per batch (the "all-reduce") in f32.

All matmul operands are bf16 (inputs cast on host; f32 PSUM accumulation)
which halves DMA traffic and runs the PE at 1 cycle/row even for narrow
diagonal tiles. Rel err vs f32 reference ~4e-3 (tolerance 2e-2).

Single fused instruction stream, engine roles:
  PE:   Q/KT/V projections, QK^T, small diag-mask bias matmuls, AV,
        output projection. One continuous stream; filler matmuls from
        neighboring phases are interleaved into every dependency gap.
  ACT:  exp (softmax numerator, fused scale), KT/Q/Y psum->sbuf copies.
  DVE:  softmax denominator accumulation, V/O psum copies, reciprocal,
        final O normalization.
  Pool: cross-partition sum of the denominator (partition_all_reduce),
        y store DMAs (SWDGE).
  SP:   all load DMAs, batched into few large transfers (HWDGE config
        serializes globally at ~630ns/DMA, so fewer+bigger is faster).

Schedule: A: Q(0) proj | B01: KT/V for s<512 | BC0: attention(t-tile 0)
with KT/V(s>=512) interleaved as fillers | C1..C3: attention(tt) with
Q(tt+1) projection + output-projection(tt-1) interleaved | coda: YO(3).
Per-head softmax normalization chains are deferred into the next head so
the PE never waits on them. Causal masking: diagonal QK/AV matmuls are
range-restricted; the 128x128 triangle gets a -1e10 bias via a tiny
identity x pattern matmul accumulated onto the logits.
"""
import math
from collections import deque

import numpy as np

import concourse.bass as bass
import concourse.bass_isa as bass_isa
import concourse.mybir as mybir
import concourse.tile as tile
from concourse import bacc
from concourse import bass_utils
from concourse.masks import make_identity

f32 = mybir.dt.float32
bf16 = mybir.dt.bfloat16
AF = mybir.ActivationFunctionType

# Problem shape (hardcoded per contract)
B, T, S, E, N, D = 2, 2048, 2048, 2048, 16, 128
N_CORES = 8
HL = 4              # heads per core
P = 128             # partitions
HD = HL * D         # 512
NE = E // P         # 16 contraction chunks
TT = 512            # t tile
NTT = T // TT       # 4
SB = 256            # phase-B s tile
NST = S // SB       # 8
NCH = 4             # e-chunks per DMA chunk tile (wq/wk/wv/xq)
SCALE = 1.0 / math.sqrt(D)

MM_LABELS = {}


def build_nc():
    nc = bacc.Bacc("TRN2", target_bir_lowering=False, debug=False)

    def mm(label, *args, **kw):
        r = nc.tensor.matmul(*args, **kw)
        MM_LABELS[r.ins.name] = label
        return r

    # DRAM tensors; all host-packed so every load is a contiguous slice.
    xq_d = nc.dram_tensor("xq", [P, NE, T], bf16, kind="ExternalInput")
    xkv_d = nc.dram_tensor("xkv", [P, NE, S], bf16, kind="ExternalInput")
    wq_d = nc.dram_tensor("wq", [P, NE, HD], bf16, kind="ExternalInput")
    wk_d = nc.dram_tensor("wk", [P, NE, HD], bf16, kind="ExternalInput")
    wv_d = nc.dram_tensor("wv", [P, NE, HD], bf16, kind="ExternalInput")
    wo_d = nc.dram_tensor("wo", [P, HL, E], bf16, kind="ExternalInput")
    y_d = nc.dram_tensor("y", [P, T // P, E], bf16, kind="ExternalOutput")

    with tile.TileContext(nc) as tc:
        with tc.tile_pool(name="persist", bufs=1) as persist:
            kt_all = persist.tile([P, HL, S], bf16)        # K^T [d, h, s]
            v_all = persist.tile([P, S // P, HD], bf16)    # V [s-in-blk, blk, hd]
            wo_all = persist.tile([P, HL, E], bf16)        # Wo [d, h, e]
            tri = persist.tile([P, P], bf16)               # -1e10 strict lower tri
            ident = persist.tile([P, P], bf16)

            with tc.tile_pool(name="init", bufs=1) as initp:
                scr = initp.tile([P, P], f32)
                nc.gpsimd.memset(scr[:], 0.0)
                # keep 0 where tj - si >= 0, else fill -1e10
                nc.gpsimd.affine_select(
                    out=scr[:], in_=scr[:],
                    compare_op=mybir.AluOpType.is_ge,
                    fill=-1e10, base=0,
                    pattern=[[1, P]], channel_multiplier=-1,
                )
                with nc.allow_low_precision(reason="-1e10 mask bias to bf16"):
                    nc.vector.tensor_copy(tri[:], scr[:])
                idf = initp.tile([P, P], f32)
                make_identity(nc, idf[:])
                with nc.allow_low_precision(reason="identity to bf16"):
                    nc.vector.tensor_copy(ident[:], idf[:])

            # PE p-state warmup: one long accumulation group of dummy
            # matmuls keeps the PE continuously busy from ~1.5us so the
            # clock is fully ramped before the first real projection.
            NWU = 30

            # ---- long-lived ring pools ----
            wqp_cm = tc.tile_pool(name="wqp", bufs=1)
            wqp = wqp_cm.__enter__()
            wq_c = [wqp.tile([P, NCH, HD], bf16, name=f"wq{c}") for c in range(NCH)]
            xqp_cm = tc.tile_pool(name="xqp", bufs=8)
            xqp = xqp_cm.__enter__()
            qtp_cm = tc.tile_pool(name="qtp", bufs=3)
            qtp = qtp_cm.__enter__()
            onp_cm = tc.tile_pool(name="onp", bufs=3)
            onp = onp_cm.__enter__()
            epp_cm = tc.tile_pool(name="epp", bufs=6)
            epp = epp_cm.__enter__()
            esp_cm = tc.tile_pool(name="esp", bufs=2)
            esp = esp_cm.__enter__()
            rbp_cm = tc.tile_pool(name="rbp", bufs=2)
            rbp = rbp_cm.__enter__()
            yp_cm = tc.tile_pool(name="yp", bufs=2)
            yp = yp_cm.__enter__()
            psl_cm = tc.tile_pool(name="psl", bufs=2, space="PSUM")
            psl = psl_cm.__enter__()
            pso_cm = tc.tile_pool(name="pso", bufs=2, space="PSUM")
            pso = pso_cm.__enter__()

            # phase-B-scoped pools
            wkvp_cm = tc.tile_pool(name="wkvp", bufs=1)
            wkvp = wkvp_cm.__enter__()
            wk_c = [wkvp.tile([P, NCH, HD], bf16, name=f"wk{c}") for c in range(NCH)]
            wv_c = [wkvp.tile([P, NCH, HD], bf16, name=f"wv{c}") for c in range(NCH)]
            xkvp_cm = tc.tile_pool(name="xkvp", bufs=2)
            xkvp = xkvp_cm.__enter__()
            pskt = psv = None  # PSUM pools opened after phase A

            xq_tiles = {}   # tt -> list of chunk tiles

            def load_xq(tt, chunks=range(NCH)):
                tiles = xq_tiles.setdefault(tt, [None] * NCH)
                chunks = [c for c in chunks if tiles[c] is None]
                for c in chunks:
                    t = xqp.tile([P, NCH, TT], bf16, tag="xq", name=f"xq{tt}_{c}")
                    nc.sync.dma_start(
                        t[:], xq_d[:, c * NCH:(c + 1) * NCH, tt * TT:(tt + 1) * TT])
                    tiles[c] = t

            xkv_tiles = {}

            def load_xkv(st):
                if st in xkv_tiles:
                    return
                t = xkvp.tile([P, NE, SB], bf16, tag="xkv", name=f"xkv{st}")
                nc.sync.dma_start(t[:], xkv_d[:, :, st * SB:(st + 1) * SB])
                xkv_tiles[st] = t

            def load_w(dst_chunks, src, chunks=range(NCH)):
                for c in chunks:
                    nc.sync.dma_start(dst_chunks[c][:],
                                      src[:, c * NCH:(c + 1) * NCH, :])

            qt_tiles = {}

            # ---------------- Phase B work generator ----------------
            def b_st(st):
                """KT/V accumulation for one s tile; yields after each matmul."""
                if st + 1 < NST:
                    load_xkv(st + 1)
                xt = xkv_tiles[st]

                def kt_group(h):
                    ps = pskt.tile([P, SB], f32, tag="pskt", name=f"psKT{st}_{h}")
                    for e in range(NE):
                        c, ce = divmod(e, NCH)
                        mm("KT", ps[:], wk_c[c][:, ce, h * D:(h + 1) * D],
                           xt[:, e, :], start=(e == 0), stop=(e == NE - 1))
                        yield
                    with nc.allow_low_precision(reason="K^T stored bf16"):
                        nc.scalar.activation(
                            kt_all[:, h, st * SB:(st + 1) * SB], ps[:], AF.Copy)

                def v_group(j):
                    ps = psv.tile([P, HD], f32, tag="psv", name=f"psV{st}_{j}")
                    for e in range(NE):
                        c, ce = divmod(e, NCH)
                        mm("V", ps[:], xt[:, e, j * P:(j + 1) * P],
                           wv_c[c][:, ce, :], start=(e == 0), stop=(e == NE - 1))
                        yield
                    with nc.allow_low_precision(reason="V stored bf16"):
                        nc.vector.tensor_copy(v_all[:, st * 2 + j, :], ps[:])

                # order: KTh0 Vj0 KTh1 KTh2 Vj1 KTh3 (rolling psum drains)
                for g in (kt_group(0), v_group(0), kt_group(1), kt_group(2),
                          v_group(1), kt_group(3)):
                    yield from g

            def b_work(sts):
                for st in sts:
                    yield from b_st(st)

            # deferred per-head normalization chains
            pending = deque()

            def drain(n=1):
                for _ in range(n):
                    while pending:
                        try:
                            next(pending[0])
                            return
                        except StopIteration:
                            pending.popleft()

            def flush_pending():
                while pending:
                    drain()

            def tail_gen(tt, h, psO, esumA, esumB, onorm):
                nc.vector.tensor_add(esumA[:], esumA[:], esumB[:])
                yield
                Rb = rbp.tile([P, TT], f32, tag="rb", name=f"rb{tt}_{h}")
                nc.gpsimd.partition_all_reduce(
                    Rb[:], esumA[:], channels=P, reduce_op=bass_isa.ReduceOp.add)
                yield
                rec = rbp.tile([P, TT], bf16, tag="rec", name=f"rec{tt}_{h}")
                with nc.allow_low_precision(reason="1/R feeds a bf16 matmul"):
                    nc.vector.reciprocal(rec[:], Rb[:])
                yield
                with nc.allow_low_precision(reason="normalized O is bf16"):
                    nc.vector.tensor_mul(onorm[:, h, :], psO[:], rec[:])
                yield

            # ---------------- filler generators ----------------
            def q_work(tt, nbanks=2):
                """Q projection for tile tt; run during tile tt-1. With
                nbanks=4 (phase A: spare banks) all heads accumulate in one
                sweep so each xq chunk is fully consumed on arrival."""
                with tc.tile_pool(name=f"psq{tt}", bufs=nbanks,
                                  space="PSUM") as psq:
                    qt = qtp.tile([P, HL, TT], bf16, tag="qt", name=f"qt{tt}")
                    hper = nbanks
                    for sweep in range(HL // hper):
                        hs = tuple(range(sweep * hper, (sweep + 1) * hper))
                        ps = [psq.tile([P, TT], f32, tag="psq", name=f"psQ{tt}_{h}")
                              for h in hs]
                        for e in range(NE):
                            c, ce = divmod(e, NCH)
                            for i, h in enumerate(hs):
                                mm("Q", ps[i][:], wq_c[c][:, ce, h * D:(h + 1) * D],
                                   xq_tiles[tt][c][:, ce, :],
                                   start=(e == 0), stop=(e == NE - 1))
                                yield
                        with nc.allow_low_precision(reason="Q stored bf16"):
                            for i, h in enumerate(hs):
                                nc.scalar.activation(qt[:, h, :], ps[i][:],
                                                     AF.Copy)
                    qt_tiles[tt] = qt

            def yo_work(tt, psys):
                """Output projection for tile tt; run during tile tt+1.
                psum->sbuf copies on DVE (ACT is kept exp-only); one batched
                store per e-chunk."""
                onr = on_tiles[tt]
                nchunk = 0
                for et in range(E // TT):
                    ysb = yp.tile([P, TT // P, TT], bf16, tag="ysb",
                                  name=f"ysb{tt}_{et}")
                    for j in range(TT // P):
                        psy = psys[nchunk % len(psys)]
                        nchunk += 1
                        psY = psy.tile([P, TT], f32, tag="psy", name="psY")
                        for h in range(HL):
                            mm("YO", psY[:], onr[:, h, j * P:(j + 1) * P],
                               wo_all[:, h, et * TT:(et + 1) * TT],
                               start=(h == 0), stop=(h == HL - 1))
                            yield
                        with nc.allow_low_precision(reason="y partial bf16"):
                            nc.vector.tensor_copy(ysb[:, j, :], psY[:])
                        if tt == NTT - 1 and et == E // TT - 1:
                            eng = nc.sync if j % 2 == 0 else nc.gpsimd
                            eng.dma_start(
                                y_d[:, tt * 4 + j, et * TT:(et + 1) * TT],
                                ysb[:, j, :])
                    if not (tt == NTT - 1 and et == E // TT - 1):
                        nc.sync.dma_start(
                            y_d[:, tt * 4:tt * 4 + 4, et * TT:(et + 1) * TT],
                            ysb[:])

            on_tiles = {}

            # ---------------- attention ----------------
            def attention(tt, filler, rate, psls=None):
                nsb = (tt + 1) * (TT // P)
                onorm = onp.tile([P, HL, TT], bf16, tag="on", name=f"on{tt}")
                on_tiles[tt] = onorm
                qt = qt_tiles[tt]
                psls = psls or [psl]
                credit = 0.0
                nl = 0

                def fill():
                    nonlocal credit
                    credit += rate
                    while credit >= 1.0 and filler is not None:
                        try:
                            next(filler)
                        except StopIteration:
                            break
                        credit -= 1.0

                for h in range(HL):
                    psO = pso.tile([P, TT], f32, tag="pso", name=f"psO{tt}_{h}")
                    # softmax denominator: two accumulators so DVE (even
                    # blocks) and Pool (odd blocks) split the adds
                    esumA = esp.tile([P, TT], f32, tag="esA", name=f"esA{tt}_{h}")
                    esumB = esp.tile([P, TT], f32, tag="esB", name=f"esB{tt}_{h}")
                    prev = None
                    for i, sb in enumerate(range(nsb)):
                        k = sb - 4 * tt
                        c0 = k * P if k >= 0 else 0
                        pslp = psls[nl % len(psls)]
                        nl += 1
                        psL = pslp.tile([P, TT], f32, tag="psl", name="psL")
                        if k >= 0:
                            mm("QK", psL[:, c0:], kt_all[:, h, sb * P:(sb + 1) * P],
                               qt[:, h, c0:], start=True, stop=False)
                            mm("MB", psL[:, c0:c0 + P], ident[:], tri[:],
                               start=False, stop=True)
                        else:
                            mm("QK", psL[:, :], kt_all[:, h, sb * P:(sb + 1) * P],
                               qt[:, h, :])
                        ep = epp.tile([P, TT], bf16, tag="ep", name="ep")
                        with nc.allow_low_precision(reason="softmax probs bf16"):
                            nc.scalar.activation(ep[:, :TT - c0], psL[:, c0:],
                                                 AF.Exp, scale=SCALE)
                        eng = nc.vector if i % 2 == 0 else nc.gpsimd
                        esum = esumA if i % 2 == 0 else esumB
                        if i == 0:
                            nc.vector.tensor_copy(esumA[:], ep[:])
                        elif i == 1:
                            if c0 > 0:
                                nc.gpsimd.memset(esumB[:, :c0], 0.0)
                            nc.gpsimd.tensor_copy(esumB[:, c0:], ep[:, :TT - c0])
                        else:
                            eng.tensor_add(esum[:, c0:], esum[:, c0:],
                                           ep[:, :TT - c0])
                        drain(1)
                        fill()
                        if prev is not None:
                            psb, pep, pc0 = prev
                            mm("AV", psO[:, pc0:],
                               v_all[:, psb, h * D:(h + 1) * D],
                               pep[:, :TT - pc0],
                               start=(psb == 0), stop=False)
                        prev = (sb, ep, c0)
                    psb, pep, pc0 = prev
                    mm("AV", psO[:, pc0:], v_all[:, psb, h * D:(h + 1) * D],
                       pep[:, :TT - pc0], start=(psb == 0), stop=True)
                    pending.append(tail_gen(tt, h, psO, esumA, esumB, onorm))

            # ---------------- DMA schedule (sync queue order) ----------------
            # phase-A operands stream first (PE start is gated on them),
            # then phase-B feed in consumption order.
            # first chunks split in halves so the first Q matmul starts
            # ~2us earlier (subtile deps unlock per half)
            nc.sync.dma_start(wq_c[0][:, :2, :], wq_d[:, 0:2, :])
            xq00 = xqp.tile([P, NCH, TT], bf16, tag="xq", name="xq0_0")
            nc.sync.dma_start(xq00[:, :2, :], xq_d[:, 0:2, :TT])
            nc.sync.dma_start(wq_c[0][:, 2:, :], wq_d[:, 2:NCH, :])
            nc.sync.dma_start(xq00[:, 2:, :], xq_d[:, 2:NCH, :TT])
            xq_tiles[0] = [xq00, None, None, None]
            for c in range(1, NCH):
                load_w(wq_c, wq_d, [c]); load_xq(0, [c])
            load_w(wk_c, wk_d, [0])
            t0 = xkvp.tile([P, NE, SB], bf16, tag="xkv", name="xkv0")
            nc.sync.dma_start(t0[:, :NE // 2, :], xkv_d[:, :NE // 2, :SB])
            nc.sync.dma_start(t0[:, NE // 2:, :], xkv_d[:, NE // 2:, :SB])
            xkv_tiles[0] = t0
            load_w(wk_c, wk_d, [1, 2, 3])
            load_w(wv_c, wv_d, [0, 1, 2, 3])
            load_xkv(1)
            # prefetch the remaining kv tiles ahead of the phase-C loads:
            # the sync queue stalls on the xkv ring waits, which is fine --
            # everything behind is needed much later.
            load_xkv(2)
            load_xkv(3)
            load_xkv(4)
            load_xkv(5)
            load_xq(1)
            load_xkv(6)
            load_xkv(7)
            nc.sync.dma_start(wo_all[:], wo_d[:, :, :])
            load_xq(2)

            # PE p-state warmup group (garbage accumulation, never read)
            psWU = psl.tile([P, TT], f32, tag="psl", name="psWU")
            for i in range(NWU):
                mm("WU", psWU[:, :P], ident[:], ident[:],
                   start=(i == 0), stop=(i == NWU - 1))

            # Phase A: Q projection for tile 0
            for _ in q_work(0, nbanks=4):
                pass

            # phase-B PSUM pools (opened after phase A's psq0 released banks)
            pskt_cm = tc.tile_pool(name="pskt", bufs=2, space="PSUM")
            pskt = pskt_cm.__enter__()
            psv_cm = tc.tile_pool(name="psv", bufs=2, space="PSUM")
            psv = psv_cm.__enter__()

            # B01: st0, st1 emitted directly (no attention to interleave yet)
            for _ in b_work(range(2)):
                pass

            # BC0: attention(0) with remaining KT/V work as fillers
            fil0 = b_work(range(2, NST))
            attention(0, fil0, rate=36.0)
            for _ in fil0:
                drain(1)

            # close phase-B pools, open psy
            psv_cm.__exit__(None, None, None)
            pskt_cm.__exit__(None, None, None)
            xkvp_cm.__exit__(None, None, None)
            wkvp_cm.__exit__(None, None, None)
            psy_cm = tc.tile_pool(name="psy", bufs=2, space="PSUM")
            psy = psy_cm.__enter__()

            # C1: leading Q(1), then attention(1) + fillers Q(2), YO(0)
            for _ in q_work(1):
                drain(1)
            load_xq(3)

            def chain(*gens):
                for g in gens:
                    yield from g

            fil1 = chain(q_work(2), q_work(3), yo_work(0, [psy]))
            attention(1, fil1, rate=6.0)
            for _ in fil1:
                drain(1)
            # tt2/tt3: psq banks are free again -> deepen QK run-ahead
            pslb_cm = tc.tile_pool(name="pslb", bufs=2, space="PSUM")
            pslb = pslb_cm.__enter__()
            fil2 = yo_work(1, [psy])
            attention(2, fil2, rate=1.33, psls=[psl, pslb])
            for _ in fil2:
                drain(1)
            fil3 = yo_work(2, [psy])
            attention(3, fil3, rate=0.85, psls=[psl, pslb])
            pslb_cm.__exit__(None, None, None)

            # coda: remaining fillers + tails + YO(3) double-buffered 4-wide
            psyb_cm = tc.tile_pool(name="psyb", bufs=2, space="PSUM")
            psyb = psyb_cm.__enter__()
            for _ in fil3:
                drain(1)
            for _ in yo_work(3, [psy, psyb]):
                drain(1)
            flush_pending()
            psyb_cm.__exit__(None, None, None)

            psy_cm.__exit__(None, None, None)
            for cm in (yp_cm, rbp_cm, esp_cm, epp_cm, onp_cm,
                       qtp_cm, xqp_cm, wqp_cm):
                cm.__exit__(None, None, None)
            pso_cm.__exit__(None, None, None)
            psl_cm.__exit__(None, None, None)

    nc.compile()
    return nc


_NC_CACHE = {}


def _get_nc(key=0):
    if key not in _NC_CACHE:
        _NC_CACHE[key] = build_nc()
    return _NC_CACHE[key]


def kernel(inputs_q, inputs_kv, Wq, Wk, Wv, Wo):
    import ml_dtypes
    bf = ml_dtypes.bfloat16

    inputs_q = np.asarray(inputs_q, dtype=np.float32)
    inputs_kv = np.asarray(inputs_kv, dtype=np.float32)
    Wq = np.asarray(Wq, dtype=np.float32)
    Wk = np.asarray(Wk, dtype=np.float32)
    Wv = np.asarray(Wv, dtype=np.float32)
    Wo = np.asarray(Wo, dtype=np.float32)

    nc = _get_nc()

    def pack_x(x):  # [T, E] -> [P, NE, T] (x^T with e split into chunks)
        return np.ascontiguousarray(
            x.T.reshape(NE, P, -1).transpose(1, 0, 2).astype(bf))

    def pack_w(W, h0):  # [E, N, D] -> [P, NE, HD] for heads [h0, h0+HL)
        Wg = W[:, h0:h0 + HL, :].reshape(E, HD)
        return np.ascontiguousarray(
            Wg.reshape(NE, P, HD).transpose(1, 0, 2).astype(bf))

    xq_b = [pack_x(inputs_q[b]) for b in range(B)]
    xkv_b = [pack_x(inputs_kv[b]) for b in range(B)]

    in_maps = []
    for c in range(N_CORES):
        b, g = divmod(c, N_CORES // B)
        h0 = g * HL
        in_maps.append({
            "xq": xq_b[b],
            "xkv": xkv_b[b],
            "wq": pack_w(Wq, h0),
            "wk": pack_w(Wk, h0),
            "wv": pack_w(Wv, h0),
            "wo": np.ascontiguousarray(
                Wo[h0:h0 + HL].transpose(1, 0, 2).astype(bf)),
        })

    res = bass_utils.run_bass_kernel_spmd(nc, in_maps, core_ids=list(range(N_CORES)))

    out = np.zeros((B, T, E), dtype=np.float32)
    for c in range(N_CORES):
        b = c // (N_CORES // B)
        yc = np.asarray(res.results[c]["y"]).astype(np.float32)  # [P, T//P, E]
        out[b] += yc.transpose(1, 0, 2).reshape(T, E)
    return out


# revision 28
# speedup vs baseline: 1.0009x; 1.0009x over previous
"""Multi-head dot-product attention (causal) on 8 TRN2 NeuronCores.

Sharding (Megatron-style per hint): batch (2) x head-groups (4 of 4 heads)
= 8 cores. Each core: q/k/v projections for its 4 heads, causal attention,
partial output projection Y_c = sum_h O_h @ Wo_h. Host sums the 4 partials
per batch (the "all-reduce") in f32.

All matmul operands are bf16 (inputs cast on host; f32 PSUM accumulation)
which halves DMA traffic and runs the PE at 1 cycle/row even for narrow
diagonal tiles. Rel err vs f32 reference ~4e-3 (tolerance 2e-2).

Single fused instruction stream, engine roles:
  PE:   Q/KT/V projections, QK^T, small diag-mask bias matmuls, AV,
        output projection. One continuous stream; filler matmuls from
        neighboring phases are interleaved into every dependency gap.
  ACT:  exp (softmax numerator, fused scale), KT/Q/Y psum->sbuf copies.
  DVE:  softmax denominator accumulation, V/O psum copies, reciprocal,
        final O normalization.
  Pool: cross-partition sum of the denominator (partition_all_reduce),
        y store DMAs (SWDGE).
  SP:   all load DMAs, batched into few large transfers (HWDGE config
        serializes globally at ~630ns/DMA, so fewer+bigger is faster).

Schedule: A: Q(0) proj | B01: KT/V for s<512 | BC0: attention(t-tile 0)
with KT/V(s>=512) interleaved as fillers | C1..C3: attention(tt) with
Q(tt+1) projection + output-projection(tt-1) interleaved | coda: YO(3).
Per-head softmax normalization chains are deferred into the next head so
the PE never waits on them. Causal masking: diagonal QK/AV matmuls are
range-restricted; the 128x128 triangle gets a -1e10 bias via a tiny
identity x pattern matmul accumulated onto the logits.
"""
import math
from collections import deque

import numpy as np

import concourse.bass as bass
import concourse.bass_isa as bass_isa
import concourse.mybir as mybir
import concourse.tile as tile
from concourse import bacc
from concourse import bass_utils
from concourse.masks import make_identity

f32 = mybir.dt.float32
bf16 = mybir.dt.bfloat16
AF = mybir.ActivationFunctionType

# Problem shape (hardcoded per contract)
B, T, S, E, N, D = 2, 2048, 2048, 2048, 16, 128
N_CORES = 8
HL = 4              # heads per core
P = 128             # partitions
HD = HL * D         # 512
NE = E // P         # 16 contraction chunks
TT = 512            # t tile
NTT = T // TT       # 4
SB = 256            # phase-B s tile
NST = S // SB       # 8
NCH = 4             # e-chunks per DMA chunk tile (wq/wk/wv/xq)
SCALE = 1.0 / math.sqrt(D)

MM_LABELS = {}


def build_nc():
    nc = bacc.Bacc("TRN2", target_bir_lowering=False, debug=False)

    def mm(label, *args, **kw):
        r = nc.tensor.matmul(*args, **kw)
        MM_LABELS[r.ins.name] = label
        return r

    # DRAM tensors; all host-packed so every load is a contiguous slice.
    xq_d = nc.dram_tensor("xq", [P, NE, T], bf16, kind="ExternalInput")
    xkv_d = nc.dram_tensor("xkv", [P, NE, S], bf16, kind="ExternalInput")
    wq_d = nc.dram_tensor("wq", [P, NE, HD], bf16, kind="ExternalInput")
    wk_d = nc.dram_tensor("wk", [P, NE, HD], bf16, kind="ExternalInput")
    wv_d = nc.dram_tensor("wv", [P, NE, HD], bf16, kind="ExternalInput")
    wo_d = nc.dram_tensor("wo", [P, HL, E], bf16, kind="ExternalInput")
    y_d = nc.dram_tensor("y", [P, T // P, E], bf16, kind="ExternalOutput")

    with tile.TileContext(nc) as tc:
        with tc.tile_pool(name="persist", bufs=1) as persist:
            kt_all = persist.tile([P, HL, S], bf16)        # K^T [d, h, s]
            v_all = persist.tile([P, S // P, HD], bf16)    # V [s-in-blk, blk, hd]
            wo_all = persist.tile([P, HL, E], bf16)        # Wo [d, h, e]
            tri = persist.tile([P, P], bf16)               # -1e10 strict lower tri
            ident = persist.tile([P, P], bf16)

            with tc.tile_pool(name="init", bufs=1) as initp:
                scr = initp.tile([P, P], f32)
                nc.gpsimd.memset(scr[:], 0.0)
                # keep 0 where tj - si >= 0, else fill -1e10
                nc.gpsimd.affine_select(
                    out=scr[:], in_=scr[:],
                    compare_op=mybir.AluOpType.is_ge,
                    fill=-1e10, base=0,
                    pattern=[[1, P]], channel_multiplier=-1,
                )
                with nc.allow_low_precision(reason="-1e10 mask bias to bf16"):
                    nc.vector.tensor_copy(tri[:], scr[:])
                idf = initp.tile([P, P], f32)
                make_identity(nc, idf[:])
                with nc.allow_low_precision(reason="identity to bf16"):
                    nc.vector.tensor_copy(ident[:], idf[:])

            # PE p-state warmup: one long accumulation group of dummy
            # matmuls keeps the PE continuously busy from ~1.5us so the
            # clock is fully ramped before the first real projection.
            NWU = 30

            # ---- long-lived ring pools ----
            wqp_cm = tc.tile_pool(name="wqp", bufs=1)
            wqp = wqp_cm.__enter__()
            wq_c = [wqp.tile([P, NCH, HD], bf16, name=f"wq{c}") for c in range(NCH)]
            xqp_cm = tc.tile_pool(name="xqp", bufs=8)
            xqp = xqp_cm.__enter__()
            qtp_cm = tc.tile_pool(name="qtp", bufs=3)
            qtp = qtp_cm.__enter__()
            onp_cm = tc.tile_pool(name="onp", bufs=3)
            onp = onp_cm.__enter__()
            epp_cm = tc.tile_pool(name="epp", bufs=6)
            epp = epp_cm.__enter__()
            esp_cm = tc.tile_pool(name="esp", bufs=2)
            esp = esp_cm.__enter__()
            rbp_cm = tc.tile_pool(name="rbp", bufs=2)
            rbp = rbp_cm.__enter__()
            yp_cm = tc.tile_pool(name="yp", bufs=2)
            yp = yp_cm.__enter__()
            psl_cm = tc.tile_pool(name="psl", bufs=2, space="PSUM")
            psl = psl_cm.__enter__()
            pso_cm = tc.tile_pool(name="pso", bufs=2, space="PSUM")
            pso = pso_cm.__enter__()

            # phase-B-scoped pools
            wkvp_cm = tc.tile_pool(name="wkvp", bufs=1)
            wkvp = wkvp_cm.__enter__()
            wk_c = [wkvp.tile([P, NCH, HD], bf16, name=f"wk{c}") for c in range(NCH)]
            wv_c = [wkvp.tile([P, NCH, HD], bf16, name=f"wv{c}") for c in range(NCH)]
            xkvp_cm = tc.tile_pool(name="xkvp", bufs=2)
            xkvp = xkvp_cm.__enter__()
            pskt = psv = None  # PSUM pools opened after phase A

            xq_tiles = {}   # tt -> list of chunk tiles

            def load_xq(tt, chunks=range(NCH)):
                tiles = xq_tiles.setdefault(tt, [None] * NCH)
                chunks = [c for c in chunks if tiles[c] is None]
                for c in chunks:
                    t = xqp.tile([P, NCH, TT], bf16, tag="xq", name=f"xq{tt}_{c}")
                    nc.sync.dma_start(
                        t[:], xq_d[:, c * NCH:(c + 1) * NCH, tt * TT:(tt + 1) * TT])
                    tiles[c] = t

            xkv_tiles = {}

            def load_xkv(st):
                if st in xkv_tiles:
                    return
                t = xkvp.tile([P, NE, SB], bf16, tag="xkv", name=f"xkv{st}")
                nc.sync.dma_start(t[:], xkv_d[:, :, st * SB:(st + 1) * SB])
                xkv_tiles[st] = t

            def load_w(dst_chunks, src, chunks=range(NCH)):
                for c in chunks:
                    nc.sync.dma_start(dst_chunks[c][:],
                                      src[:, c * NCH:(c + 1) * NCH, :])

            qt_tiles = {}

            # ---------------- Phase B work generator ----------------
            def b_st(st):
                """KT/V accumulation for one s tile; yields after each matmul."""
                if st + 1 < NST:
                    load_xkv(st + 1)
                xt = xkv_tiles[st]

                def kt_group(h):
                    ps = pskt.tile([P, SB], f32, tag="pskt", name=f"psKT{st}_{h}")
                    for e in range(NE):
                        c, ce = divmod(e, NCH)
                        mm("KT", ps[:], wk_c[c][:, ce, h * D:(h + 1) * D],
                           xt[:, e, :], start=(e == 0), stop=(e == NE - 1))
                        yield
                    with nc.allow_low_precision(reason="K^T stored bf16"):
                        nc.scalar.activation(
                            kt_all[:, h, st * SB:(st + 1) * SB], ps[:], AF.Copy)

                def v_group(j):
                    ps = psv.tile([P, HD], f32, tag="psv", name=f"psV{st}_{j}")
                    for e in range(NE):
                        c, ce = divmod(e, NCH)
                        mm("V", ps[:], xt[:, e, j * P:(j + 1) * P],
                           wv_c[c][:, ce, :], start=(e == 0), stop=(e == NE - 1))
                        yield
                    with nc.allow_low_precision(reason="V stored bf16"):
                        nc.vector.tensor_copy(v_all[:, st * 2 + j, :], ps[:])

                # order: KTh0 Vj0 KTh1 KTh2 Vj1 KTh3 (rolling psum drains)
                for g in (kt_group(0), v_group(0), kt_group(1), kt_group(2),
                          v_group(1), kt_group(3)):
                    yield from g

            def b_work(sts):
                for st in sts:
                    yield from b_st(st)

            # deferred per-head normalization chains
            pending = deque()

            def drain(n=1):
                for _ in range(n):
                    while pending:
                        try:
                            next(pending[0])
                            return
                        except StopIteration:
                            pending.popleft()

            def flush_pending():
                while pending:
                    drain()

            def tail_gen(tt, h, psO, esumA, esumB, onorm):
                nc.vector.tensor_add(esumA[:], esumA[:], esumB[:])
                yield
                Rb = rbp.tile([P, TT], f32, tag="rb", name=f"rb{tt}_{h}")
                nc.gpsimd.partition_all_reduce(
                    Rb[:], esumA[:], channels=P, reduce_op=bass_isa.ReduceOp.add)
                yield
                rec = rbp.tile([P, TT], bf16, tag="rec", name=f"rec{tt}_{h}")
                with nc.allow_low_precision(reason="1/R feeds a bf16 matmul"):
                    nc.vector.reciprocal(rec[:], Rb[:])
                yield
                with nc.allow_low_precision(reason="normalized O is bf16"):
                    nc.vector.tensor_mul(onorm[:, h, :], psO[:], rec[:])
                yield

            # ---------------- filler generators ----------------
            def q_work(tt, nbanks=2):
                """Q projection for tile tt; run during tile tt-1. With
                nbanks=4 (phase A: spare banks) all heads accumulate in one
                sweep so each xq chunk is fully consumed on arrival."""
                with tc.tile_pool(name=f"psq{tt}", bufs=nbanks,
                                  space="PSUM") as psq:
                    qt = qtp.tile([P, HL, TT], bf16, tag="qt", name=f"qt{tt}")
                    hper = nbanks
                    for sweep in range(HL // hper):
                        hs = tuple(range(sweep * hper, (sweep + 1) * hper))
                        ps = [psq.tile([P, TT], f32, tag="psq", name=f"psQ{tt}_{h}")
                              for h in hs]
                        for e in range(NE):
                            c, ce = divmod(e, NCH)
                            for i, h in enumerate(hs):
                                mm("Q", ps[i][:], wq_c[c][:, ce, h * D:(h + 1) * D],
                                   xq_tiles[tt][c][:, ce, :],
                                   start=(e == 0), stop=(e == NE - 1))
                                yield
                        with nc.allow_low_precision(reason="Q stored bf16"):
                            for i, h in enumerate(hs):
                                nc.scalar.activation(qt[:, h, :], ps[i][:],
                                                     AF.Copy)
                    qt_tiles[tt] = qt

            def yo_work(tt, psys):
                """Output projection for tile tt; run during tile tt+1.
                psum->sbuf copies on DVE (ACT is kept exp-only); one batched
                store per e-chunk."""
                onr = on_tiles[tt]
                nchunk = 0
                for et in range(E // TT):
                    ysb = yp.tile([P, TT // P, TT], bf16, tag="ysb",
                                  name=f"ysb{tt}_{et}")
                    for j in range(TT // P):
                        psy = psys[nchunk % len(psys)]
                        nchunk += 1
                        psY = psy.tile([P, TT], f32, tag="psy", name="psY")
                        for h in range(HL):
                            mm("YO", psY[:], onr[:, h, j * P:(j + 1) * P],
                               wo_all[:, h, et * TT:(et + 1) * TT],
                               start=(h == 0), stop=(h == HL - 1))
                            yield
                        with nc.allow_low_precision(reason="y partial bf16"):
                            nc.vector.tensor_copy(ysb[:, j, :], psY[:])
                        if tt == NTT - 1 and et == E // TT - 1:
                            nc.sync.dma_start(
                                y_d[:, tt * 4 + j, et * TT:(et + 1) * TT],
                                ysb[:, j, :])
                    if not (tt == NTT - 1 and et == E // TT - 1):
                        nc.sync.dma_start(
                            y_d[:, tt * 4:tt * 4 + 4, et * TT:(et + 1) * TT],
                            ysb[:])

            on_tiles = {}

            # ---------------- attention ----------------
            def attention(tt, filler, rate, psls=None):
                nsb = (tt + 1) * (TT // P)
                onorm = onp.tile([P, HL, TT], bf16, tag="on", name=f"on{tt}")
                on_tiles[tt] = onorm
                qt = qt_tiles[tt]
                psls = psls or [psl]
                credit = 0.0
                nl = 0

                def fill():
                    nonlocal credit
                    credit += rate
                    while credit >= 1.0 and filler is not None:
                        try:
                            next(filler)
                        except StopIteration:
                            break
                        credit -= 1.0

                for h in range(HL):
                    psO = pso.tile([P, TT], f32, tag="pso", name=f"psO{tt}_{h}")
                    # softmax denominator: two accumulators so DVE (even
                    # blocks) and Pool (odd blocks) split the adds
                    esumA = esp.tile([P, TT], f32, tag="esA", name=f"esA{tt}_{h}")
                    esumB = esp.tile([P, TT], f32, tag="esB", name=f"esB{tt}_{h}")
                    prev = None
                    for i, sb in enumerate(range(nsb)):
                        k = sb - 4 * tt
                        c0 = k * P if k >= 0 else 0
                        pslp = psls[nl % len(psls)]
                        nl += 1
                        psL = pslp.tile([P, TT], f32, tag="psl", name="psL")
                        if k >= 0:
                            mm("QK", psL[:, c0:], kt_all[:, h, sb * P:(sb + 1) * P],
                               qt[:, h, c0:], start=True, stop=False)
                            mm("MB", psL[:, c0:c0 + P], ident[:], tri[:],
                               start=False, stop=True)
                        else:
                            mm("QK", psL[:, :], kt_all[:, h, sb * P:(sb + 1) * P],
                               qt[:, h, :])
                        ep = epp.tile([P, TT], bf16, tag="ep", name="ep")
                        with nc.allow_low_precision(reason="softmax probs bf16"):
                            nc.scalar.activation(ep[:, :TT - c0], psL[:, c0:],
                                                 AF.Exp, scale=SCALE)
                        eng = nc.vector if i % 2 == 0 else nc.gpsimd
                        esum = esumA if i % 2 == 0 else esumB
                        if i == 0:
                            nc.vector.tensor_copy(esumA[:], ep[:])
                        elif i == 1:
                            if c0 > 0:
                                nc.gpsimd.memset(esumB[:, :c0], 0.0)
                            nc.gpsimd.tensor_copy(esumB[:, c0:], ep[:, :TT - c0])
                        else:
                            eng.tensor_add(esum[:, c0:], esum[:, c0:],
                                           ep[:, :TT - c0])
                        drain(1)
                        fill()
                        if prev is not None:
                            psb, pep, pc0 = prev
                            mm("AV", psO[:, pc0:],
                               v_all[:, psb, h * D:(h + 1) * D],
                               pep[:, :TT - pc0],
                               start=(psb == 0), stop=False)
                        prev = (sb, ep, c0)
                    psb, pep, pc0 = prev
                    mm("AV", psO[:, pc0:], v_all[:, psb, h * D:(h + 1) * D],
                       pep[:, :TT - pc0], start=(psb == 0), stop=True)
                    pending.append(tail_gen(tt, h, psO, esumA, esumB, onorm))

            # ---------------- DMA schedule (sync queue order) ----------------
            # phase-A operands stream first (PE start is gated on them),
            # then phase-B feed in consumption order.
            # first chunks split in halves so the first Q matmul starts
            # ~2us earlier (subtile deps unlock per half)
            nc.sync.dma_start(wq_c[0][:, :2, :], wq_d[:, 0:2, :])
            xq00 = xqp.tile([P, NCH, TT], bf16, tag="xq", name="xq0_0")
            nc.sync.dma_start(xq00[:, :2, :], xq_d[:, 0:2, :TT])
            nc.sync.dma_start(wq_c[0][:, 2:, :], wq_d[:, 2:NCH, :])
            nc.sync.dma_start(xq00[:, 2:, :], xq_d[:, 2:NCH, :TT])
            xq_tiles[0] = [xq00, None, None, None]
            for c in range(1, NCH):
                load_w(wq_c, wq_d, [c]); load_xq(0, [c])
            load_w(wk_c, wk_d, [0])
            t0 = xkvp.tile([P, NE, SB], bf16, tag="xkv", name="xkv0")
            nc.sync.dma_start(t0[:, :NE // 2, :], xkv_d[:, :NE // 2, :SB])
            nc.sync.dma_start(t0[:, NE // 2:, :], xkv_d[:, NE // 2:, :SB])
            xkv_tiles[0] = t0
            load_w(wk_c, wk_d, [1, 2])
            for ce in range(NCH):
                nc.sync.dma_start(wk_c[3][:, ce:ce + 1, :],
                                  wk_d[:, 3 * NCH + ce:3 * NCH + ce + 1, :])
            load_w(wv_c, wv_d, [0, 1, 2, 3])
            load_xkv(1)
            # prefetch the remaining kv tiles ahead of the phase-C loads:
            # the sync queue stalls on the xkv ring waits, which is fine --
            # everything behind is needed much later.
            load_xkv(2)
            load_xkv(3)
            load_xkv(4)
            load_xkv(5)
            load_xq(1)
            load_xkv(6)
            load_xkv(7)
            nc.sync.dma_start(wo_all[:], wo_d[:, :, :])
            load_xq(2)

            # PE p-state warmup group (garbage accumulation, never read)
            psWU = psl.tile([P, TT], f32, tag="psl", name="psWU")
            for i in range(NWU):
                mm("WU", psWU[:, :P], ident[:], ident[:],
                   start=(i == 0), stop=(i == NWU - 1))

            # Phase A: Q projection for tile 0
            for _ in q_work(0, nbanks=4):
                pass

            # phase-B PSUM pools (opened after phase A's psq0 released banks)
            pskt_cm = tc.tile_pool(name="pskt", bufs=2, space="PSUM")
            pskt = pskt_cm.__enter__()
            psv_cm = tc.tile_pool(name="psv", bufs=2, space="PSUM")
            psv = psv_cm.__enter__()

            # B01: st0, st1 emitted directly (no attention to interleave yet)
            for _ in b_work(range(2)):
                pass

            # BC0: attention(0) with remaining KT/V work as fillers
            fil0 = b_work(range(2, NST))
            attention(0, fil0, rate=36.0)
            for _ in fil0:
                drain(1)

            # close phase-B pools, open psy
            psv_cm.__exit__(None, None, None)
            pskt_cm.__exit__(None, None, None)
            xkvp_cm.__exit__(None, None, None)
            wkvp_cm.__exit__(None, None, None)
            psy_cm = tc.tile_pool(name="psy", bufs=2, space="PSUM")
            psy = psy_cm.__enter__()

            # C1: leading Q(1), then attention(1) + fillers Q(2), YO(0)
            for _ in q_work(1):
                drain(1)
            load_xq(3)

            def chain(*gens):
                for g in gens:
                    yield from g

            fil1 = chain(q_work(2), q_work(3), yo_work(0, [psy]))
            attention(1, fil1, rate=6.0)
            for _ in fil1:
                drain(1)
            # tt2/tt3: psq banks are free again -> deepen QK run-ahead
            pslb_cm = tc.tile_pool(name="pslb", bufs=2, space="PSUM")
            pslb = pslb_cm.__enter__()
            fil2 = yo_work(1, [psy])
            attention(2, fil2, rate=1.33, psls=[psl, pslb])
            for _ in fil2:
                drain(1)
            fil3 = yo_work(2, [psy])
            attention(3, fil3, rate=0.85, psls=[psl, pslb])
            pslb_cm.__exit__(None, None, None)

            # coda: remaining fillers + tails + YO(3) double-buffered 4-wide
            psyb_cm = tc.tile_pool(name="psyb", bufs=2, space="PSUM")
            psyb = psyb_cm.__enter__()
            for _ in fil3:
                drain(1)
            for _ in yo_work(3, [psy, psyb]):
                drain(1)
            flush_pending()
            psyb_cm.__exit__(None, None, None)

            psy_cm.__exit__(None, None, None)
            for cm in (yp_cm, rbp_cm, esp_cm, epp_cm, onp_cm,
                       qtp_cm, xqp_cm, wqp_cm):
                cm.__exit__(None, None, None)
            pso_cm.__exit__(None, None, None)
            psl_cm.__exit__(None, None, None)

    nc.compile()
    return nc


_NC_CACHE = {}


def _get_nc(key=0):
    if key not in _NC_CACHE:
        _NC_CACHE[key] = build_nc()
    return _NC_CACHE[key]


def kernel(inputs_q, inputs_kv, Wq, Wk, Wv, Wo):
    import ml_dtypes
    bf = ml_dtypes.bfloat16

    inputs_q = np.asarray(inputs_q, dtype=np.float32)
    inputs_kv = np.asarray(inputs_kv, dtype=np.float32)
    Wq = np.asarray(Wq, dtype=np.float32)
    Wk = np.asarray(Wk, dtype=np.float32)
    Wv = np.asarray(Wv, dtype=np.float32)
    Wo = np.asarray(Wo, dtype=np.float32)

    nc = _get_nc()

    def pack_x(x):  # [T, E] -> [P, NE, T] (x^T with e split into chunks)
        return np.ascontiguousarray(
            x.T.reshape(NE, P, -1).transpose(1, 0, 2).astype(bf))

    def pack_w(W, h0):  # [E, N, D] -> [P, NE, HD] for heads [h0, h0+HL)
        Wg = W[:, h0:h0 + HL, :].reshape(E, HD)
        return np.ascontiguousarray(
            Wg.reshape(NE, P, HD).transpose(1, 0, 2).astype(bf))

    xq_b = [pack_x(inputs_q[b]) for b in range(B)]
    xkv_b = [pack_x(inputs_kv[b]) for b in range(B)]

    in_maps = []
    for c in range(N_CORES):
        b, g = divmod(c, N_CORES // B)
        h0 = g * HL
        in_maps.append({
            "xq": xq_b[b],
            "xkv": xkv_b[b],
            "wq": pack_w(Wq, h0),
            "wk": pack_w(Wk, h0),
            "wv": pack_w(Wv, h0),
            "wo": np.ascontiguousarray(
                Wo[h0:h0 + HL].transpose(1, 0, 2).astype(bf)),
        })

    res = bass_utils.run_bass_kernel_spmd(nc, in_maps, core_ids=list(range(N_CORES)))

    out = np.zeros((B, T, E), dtype=np.float32)
    for c in range(N_CORES):
        b = c // (N_CORES // B)
        yc = np.asarray(res.results[c]["y"]).astype(np.float32)  # [P, T//P, E]
        out[b] += yc.transpose(1, 0, 2).reshape(T, E)
    return out
